# revision 1
# baseline (speedup 1.0000x reference)
"""Trainium2 Bass kernel for CapsuleLayer dynamic routing (8-core SPMD).

Strategy: shard the 2048 input capsules (n) across 8 cores. Each core builds
u_hat = einsum('bni,nio->bno') for its n-slice with W as the PE-stationary
operand so the PSUM output lands with (i4,j32) on partitions -- the native
layout for the routing b-update contraction over j. The o-contraction over n
runs on an n-partition view produced by DMA xbar transposes. Routing's
per-iteration global sum over n is an AllReduce of the tiny [32,32,32] o.
"""
import sys

sys.path.insert(0, "/opt/trn_rl_repo")

import numpy as np
import ml_dtypes

B = 32            # batch
N_TOTAL = 2048    # input capsules
KD = 16           # input capsule dim
NCAP = 32         # output capsules (i)
DIM = 32          # output capsule dim (j)
O = NCAP * DIM    # 1024
NUM_CORES = 8
NL = N_TOTAL // NUM_CORES   # 256 local n
G = NL // 8                 # 32 groups of 8 n
OC = O // 128               # 8 o-chunks
EPS_K = 1e-7
EPS_L2 = 1e-12

_PROG = {}


def _set_dims(ap, dims, offset=None):
    c = ap.copy()
    v = c.ap
    while len(v) > len(dims):
        v.pop()
    while len(v) < len(dims):
        v.insert(0, [0, 1])
    for k, d in enumerate(dims):
        v[k] = list(d)
    if offset is not None:
        c.offset = offset
    return c


def build_program(num_cores):
    import concourse.bass as bass
    import concourse.mybir as mybir
    from concourse import bacc, tile
    from concourse.tile import add_dep_helper

    f32 = mybir.dt.float32
    bf16 = mybir.dt.bfloat16
    AX = mybir.AxisListType
    OP = mybir.AluOpType
    AF = mybir.ActivationFunctionType

    nc = bacc.Bacc("TRN2", target_bir_lowering=False, num_devices=num_cores)
    rg = [list(range(num_cores))]

    wl_d = nc.dram_tensor("wl", [G, OC, 128, 128], bf16, kind="ExternalInput")
    ubd_d = nc.dram_tensor("ubd", [G, 128, 256], bf16, kind="ExternalInput")
    upl_d = nc.dram_tensor("upl", [G, 128, B], bf16, kind="ExternalInput")
    out_d = nc.dram_tensor("out", [B, O], f32, kind="ExternalOutput")

    with tile.TileContext(nc) as tc:
        with (
            tc.tile_pool(name="pers", bufs=1) as pers,
            tc.tile_pool(name="dram", bufs=1, space="DRAM") as dram,
            tc.tile_pool(name="ps_main", bufs=2, space="PSUM") as ps_main,
            tc.tile_pool(name="small", bufs=1) as small,
        ):
            u_hat = pers.tile([128, B, OC, NL], bf16, tag="u_hat")
            o_acc = pers.tile([DIM, B, NCAP], f32, tag="o_acc")
            tr_scr = pers.tile([128, 1024], f32, tag="tr_scr")
            o0_sb = pers.tile([128, OC, B], f32, tag="o0_sb")

            onrm = small.tile([B, O], f32, tag="onrm")
            onl = small.tile([B, O], f32, tag="onl")
            s2 = small.tile([B, NCAP], f32, tag="s2")
            s2b = small.tile([B, NCAP], f32, tag="s2b")
            s2c = small.tile([B, NCAP], f32, tag="s2c")
            rinv = small.tile([B, NCAP], f32, tag="rinv")
            mx = small.tile([128, 2 * B], f32, tag="mx")
            sm = small.tile([128, 2 * B], f32, tag="sm")
            smr = small.tile([128, 2 * B], f32, tag="smr")

            cc_in = [
                dram.tile([B, O], f32, tag=f"cc_in{t}", name=f"cc_in{t}")
                for t in range(3)
            ]
            cc_out = [
                dram.tile([B, O], f32, tag=f"cc_out{t}", name=f"cc_out{t}")
                for t in range(3)
            ]
            o_dram = dram.tile([B, O], f32, tag="o_dram", name="o_dram")

            # ---------------- Phase A: load + build u_hat + o0 ----------------
            with (
                tc.tile_pool(name="bpool", bufs=1) as bpool,
                tc.tile_pool(name="wpool", bufs=2) as wpool,
                tc.tile_pool(name="ps_o0", bufs=1, space="PSUM") as ps_o0,
            ):
                ubd_sb = bpool.tile([128, G, 256], bf16, tag="ubd_sb")
                upl_sb = bpool.tile([128, G, B], bf16, tag="upl_sb")
                nc.sync.dma_start(ubd_sb[:], ubd_d[:].rearrange("g p c -> p g c"))
                nc.sync.dma_start(upl_sb[:], upl_d[:].rearrange("g p c -> p g c"))

                for oc in range(OC):
                    wl_oc = wpool.tile([128, G, 128], bf16, tag="wl_oc")
                    nc.sync.dma_start(
                        wl_oc[:], wl_d[:, oc, :, :].rearrange("g p c -> p g c")
                    )
                    o0ps = ps_o0.tile([128, B], f32, tag="o0ps")
                    for g4 in range(G // 4):
                        ps = ps_main.tile([128, 1024], f32, tag="ps")
                        for gh in range(4):
                            g = g4 * 4 + gh
                            nc.tensor.matmul(
                                ps[:, gh * 256:(gh + 1) * 256],
                                wl_oc[:, g, :], ubd_sb[:, g, :],
                                start=True, stop=True,
                                skip_group_check=True,
                            )
                            nc.tensor.matmul(
                                o0ps[:], wl_oc[:, g, :], upl_sb[:, g, :],
                                start=(g == 0), stop=(g == G - 1),
                                skip_group_check=True,
                            )
                        # psum cols (gh, b, n8) -> u_hat[:, :, oc, g4*32:+32]
                        g = g4 * 4
                        dst = _set_dims(
                            u_hat[:, :, oc, 0],
                            [[B * OC * NL, 128], [8, 4], [OC * NL, B], [1, 8]],
                            offset=oc * NL + g * 8,
                        )
                        src = ps[:].rearrange("p (h b n) -> p h b n", h=4, b=B)
                        if g4 % 2 == 0:
                            nc.scalar.copy(dst, src)
                        else:
                            nc.vector.tensor_copy(dst, src)
                    nc.scalar.copy(o0_sb[:, oc, :], o0ps[:])

            # o0 partial -> dram bounce: cc_in0[b, oc*128+p] = o0_sb[p, oc, b]
            for oc in range(OC):
                src = o0_sb[:, oc, :]
                dst = _set_dims(
                    cc_in[0][:], [[1, 128], [O, B]], offset=oc * 128
                )
                nc.sync.dma_start(dst, src)

            def all_reduce(t):
                if num_cores == 1:
                    nc.gpsimd.dma_start(cc_out[t][:], cc_in[t][:])
                else:
                    nc.gpsimd.collective_compute(
                        "AllReduce", OP.add, replica_groups=rg,
                        ins=[cc_in[t][:].opt()], outs=[cc_out[t][:].opt()],
                    )

            all_reduce(0)

            # ---------------- routing iterations ----------------
            with (
                tc.tile_pool(name="rout", bufs=1) as rout,
                tc.tile_pool(name="ring", bufs=2) as ring,
                tc.tile_pool(name="tring", bufs=2) as tring,
                tc.tile_pool(name="ps_b", bufs=3, space="PSUM") as ps_b,
            ):
                OB2P = B * OC * NCAP     # obd2 pitch (8192)
                OTP = B * OC             # o_tmp pitch (256)
                o_tmp = rout.tile([128, B, OC], f32, tag="o_tmp")
                obd2 = rout.tile([128, OB2P], bf16, tag="obd2")
                blog = rout.tile([128, B, 2, NCAP], f32, tag="blog")
                c_sb = rout.tile([128, B, 2, NCAP], bf16, tag="c_sb")
                sthi = rout.tile([NCAP, 8, 256], bf16, tag="sthi")
                stlo = rout.tile([NCAP, 8, 256], bf16, tag="stlo")

                # zero once; the mask copies overwrite the same cols every iter
                ms2 = nc.gpsimd.memset(obd2[:], 0.0)
                memsets = [ms2]

                for t in range(3):
                    final = t == 2
                    ji = t > 0  # cc[0] is [b,(i,j)]; cc[1:] are [j,(b,i)]
                    # ---- load global o, normalize (l2 for t<2, squash at t=2)
                    if not ji:
                        nc.sync.dma_start(onrm[:], cc_out[t][:])
                    else:
                        # onrm[b, j*32+i] = cc_out[j, b*32+i]
                        dstL = _set_dims(
                            onrm[:], [[O, B], [NCAP, DIM], [1, NCAP]]
                        )
                        srcL = _set_dims(
                            cc_out[t][:], [[NCAP, B], [O, DIM], [1, NCAP]]
                        )
                        nc.sync.dma_start(dstL, srcL)
                    nc.scalar.square(onl[:], onrm[:])
                    if not ji:
                        red_in = onl[:].rearrange("b (i j) -> b i j", i=NCAP)
                    else:
                        red_in = _set_dims(
                            onl[:], [[O, B], [1, NCAP], [NCAP, DIM]]
                        )
                    nc.vector.tensor_reduce(s2[:], red_in, axis=AX.X, op=OP.add)
                    if not final:
                        nc.vector.tensor_scalar_max(s2b[:], s2[:], EPS_L2)
                        nc.scalar.sqrt(s2c[:], s2b[:])
                        nc.vector.reciprocal(rinv[:], s2c[:])
                    else:
                        # squash scale = s2 / ((1+s2) * sqrt(s2+eps))
                        nc.vector.tensor_scalar_add(s2b[:], s2[:], EPS_K)
                        nc.scalar.sqrt(s2b[:], s2b[:])
                        nc.vector.tensor_scalar_add(s2c[:], s2[:], 1.0)
                        nc.vector.tensor_mul(s2c[:], s2c[:], s2b[:])
                        nc.vector.reciprocal(s2b[:], s2c[:])
                        nc.vector.tensor_mul(rinv[:], s2b[:], s2[:])
                    if not ji:
                        sc_b = _set_dims(
                            rinv[:], [[NCAP, B], [1, NCAP], [0, DIM]]
                        )
                        nc.vector.tensor_mul(
                            onl[:].rearrange("b (i j) -> b i j", i=NCAP),
                            onrm[:].rearrange("b (i j) -> b i j", i=NCAP),
                            sc_b,
                        )
                    else:
                        # loops (b, j, i); onl written in (i,j) order at t=2
                        sc_b = _set_dims(
                            rinv[:], [[NCAP, B], [0, DIM], [1, NCAP]]
                        )
                        in_ji = _set_dims(
                            onrm[:], [[O, B], [NCAP, DIM], [1, NCAP]]
                        )
                        if final:
                            out_v = _set_dims(
                                onl[:], [[O, B], [1, DIM], [DIM, NCAP]]
                            )
                        else:
                            out_v = _set_dims(
                                onl[:], [[O, B], [NCAP, DIM], [1, NCAP]]
                            )
                        nc.vector.tensor_mul(out_v, in_ji, sc_b)
                    if final:
                        nc.sync.dma_start(out_d[:], onl[:])
                        break

                    # ---- scatter normalized o into block-diag (cast to bf16)
                    # bounce through DRAM: SBUF DMA APs need partition dim first
                    if not ji:
                        nc.sync.dma_start(o_dram[:], onl[:])
                    else:
                        # o_dram[j, b*32+i] = onl[b, j*32+i]
                        dstJ = _set_dims(
                            o_dram[:], [[NCAP, B], [O, DIM], [1, NCAP]]
                        )
                        srcJ = _set_dims(
                            onl[:], [[O, B], [NCAP, DIM], [1, NCAP]]
                        )
                        nc.sync.dma_start(dstJ, srcJ)
                    # o_tmp[p=i4*32+j, b*8+oc] = o(b, oc*4+i4, j); one DMA per i4
                    for i4 in range(4):
                        if not ji:
                            srcd = _set_dims(
                                o_dram[:], [[1, DIM], [128, 256]],
                                offset=i4 * DIM,
                            )
                        else:
                            srcd = _set_dims(
                                o_dram[:], [[O, DIM], [4, 256]], offset=i4
                            )
                        dstd = _set_dims(
                            o_tmp[:], [[OTP, DIM], [1, 256]],
                            offset=(32 * i4) * OTP,
                        )
                        nc.sync.dma_start(dstd, srcd)
                    # masked strided copies into obd2:
                    # obd2[32m+j, b*256 + oc*36 + m] = o_tmp[32m+j, b*8+oc]
                    for m in range(4):
                        src_e = _set_dims(
                            o_tmp[:], [[OTP, DIM], [OC, B], [1, OC]],
                            offset=(32 * m) * OTP,
                        )
                        dst_e = _set_dims(
                            obd2[:],
                            [[OB2P, DIM], [OC * NCAP, B], [NCAP + 4, OC]],
                            offset=(32 * m) * OB2P + m,
                        )
                        if m % 2 == 0:
                            ec = nc.vector.tensor_copy(dst_e, src_e)
                        else:
                            ec = nc.scalar.copy(dst_e, src_e)
                        if t == 0:
                            for ms in memsets:
                                add_dep_helper(
                                    ec.ins, ms.ins, sync=True,
                                    reason="mask copy after memset",
                                )

                    # ---- b-update: per b, 8 oc-matmuls accumulate [32, 256]
                    for b in range(B):
                        psb = ps_b.tile([NCAP, 256], f32, tag="psb")
                        for oc in range(OC):
                            lhs = _set_dims(
                                obd2[:], [[OB2P, 128], [1, NCAP]],
                                offset=b * OC * NCAP + oc * NCAP,
                            )
                            nc.tensor.matmul(
                                psb[:], lhs, u_hat[:, b, oc, :],
                                start=(oc == 0), stop=(oc == OC - 1),
                            )
                        bg, b8 = b >> 3, b & 7
                        nc.scalar.copy(sthi[:, b8, :], psb[:])
                        nc.vector.tensor_sub(stlo[:, b8, :], psb[:], sthi[:, b8, :])
                        if b8 == 7:
                            # transpose [32 i, 2048 (b8,n)] -> [128 nl, (b8,nh), 32 i]
                            thi = tring.tile([128, 16, NCAP], bf16, tag="thi")
                            tlo = tring.tile([128, 16, NCAP], bf16, tag="tlo")
                            nc.sync.dma_start_transpose(
                                thi[:], sthi[:].rearrange("p a n -> p (a n)")
                            )
                            nc.sync.dma_start_transpose(
                                tlo[:], stlo[:].rearrange("p a n -> p (a n)")
                            )
                            nc.vector.tensor_add(
                                blog[:, bg * 8:(bg + 1) * 8, :, :],
                                thi[:].rearrange("p (b h) i -> p b h i", b=8),
                                tlo[:].rearrange("p (b h) i -> p b h i", b=8),
                            )

                    # ---- softmax over i on blog [p=nl, (b, nh, i)]
                    nc.vector.tensor_reduce(mx[:], blog[:], axis=AX.X, op=OP.max)
                    mxb = _set_dims(
                        mx[:], [[2 * B, 128], [1, 2 * B], [0, NCAP]]
                    )
                    blog3 = blog[:].rearrange("p b h i -> p (b h) i")
                    nc.vector.tensor_sub(blog3, blog3, mxb)
                    nc.scalar.activation(blog[:], blog[:], AF.Exp)
                    nc.vector.tensor_reduce(sm[:], blog[:], axis=AX.X, op=OP.add)
                    nc.vector.reciprocal(smr[:], sm[:])
                    smb = _set_dims(
                        smr[:], [[2 * B, 128], [1, 2 * B], [0, NCAP]]
                    )
                    nc.vector.tensor_mul(
                        c_sb[:].rearrange("p b h i -> p (b h) i"), blog3, smb
                    )

                    # ---- o-pass: xbar-transpose u_hat per 2b, matmul with c
                    for b in range(B):
                        cg = b & 3
                        if cg == 0:
                            pso = ps_main.tile([128, 1024], f32, tag="ps")
                        if b % 2 == 0:
                            uht = ring.tile([128, 32, 128], bf16, tag="uht")
                            nc.sync.dma_start_transpose(
                                uht[:],
                                u_hat[:, b:b + 2, :, :].rearrange(
                                    "p b a n -> p (b a n)"
                                ),
                            )
                        b1 = b & 1
                        for nh in range(2):
                            lhs = c_sb[:, b, nh, :]
                            for oh in range(2):
                                rhs = _set_dims(
                                    uht[:],
                                    [[32 * 128, 128], [256, 4], [1, 128]],
                                    offset=(16 * b1 + 8 * oh + nh) * 128,
                                )
                                nc.tensor.matmul(
                                    pso[32 * cg:32 * cg + 32,
                                        oh * 512:(oh + 1) * 512],
                                    lhs, rhs,
                                    start=(nh == 0), stop=(nh == 1),
                                    tile_position=(0, 32 * cg),
                                    skip_group_check=True,
                                )
                        if cg == 3:
                            # 32x32 block transpose; diag becomes stride-33 cols
                            nc.vector.transpose(tr_scr[:], pso[:])
                            for c2 in range(4):
                                bb = b - 3 + c2
                                diag = _set_dims(
                                    tr_scr[:], [[1024, 32], [33, DIM]],
                                    offset=(32 * c2) * 1024,
                                )
                                if c2 % 2 == 0:
                                    nc.scalar.copy(o_acc[:, bb, :], diag)
                                else:
                                    nc.vector.tensor_copy(o_acc[:, bb, :], diag)

                    # o_acc [j, b, i] -> cc_in[t+1] (ji layout, same shape)
                    nc.sync.dma_start(cc_in[t + 1][:], o_acc[:])
                    all_reduce(t + 1)

    nc.compile()
    return nc


def host_prep(u_vecs, W, core):
    ns = slice(core * NL, (core + 1) * NL)
    Wc = np.asarray(W[ns], dtype=np.float32)             # [NL, 16, 1024]
    uc = np.asarray(u_vecs[:, ns, :], dtype=np.float32)  # [B, NL, 16]
    bf = ml_dtypes.bfloat16

    wl = (
        Wc.reshape(G, 8, KD, OC, 128)
        .transpose(0, 3, 1, 2, 4)
        .reshape(G, OC, 128, 128)
        .astype(bf)
    )
    tmp = uc.transpose(1, 2, 0).reshape(G, 8, KD, B)     # [g, n8, k, b]
    ubd = np.zeros((G, 8, KD, B, 8), dtype=np.float32)
    for n8 in range(8):
        ubd[:, n8, :, :, n8] = tmp[:, n8]
    ubd = ubd.reshape(G, 128, B * 8).astype(bf)
    upl = tmp.reshape(G, 128, B).astype(bf)
    return {"wl": wl, "ubd": ubd, "upl": upl}


def kernel(u_vecs, W):
    from concourse import bass_utils

    if "prog" not in _PROG:
        _PROG["prog"] = build_program(NUM_CORES)
    nc = _PROG["prog"]
    in_maps = [host_prep(u_vecs, W, c) for c in range(NUM_CORES)]
    res = bass_utils.run_bass_kernel_spmd(
        nc, in_maps, core_ids=list(range(NUM_CORES))
    )
    out = np.asarray(res.results[0]["out"], dtype=np.float32)
    return out.reshape(B, NCAP, DIM)



# revision 8
# speedup vs baseline: 2.9098x; 2.9098x over previous
"""Trainium2 Bass kernel for CapsuleLayer dynamic routing (8-core SPMD).

Strategy: shard the 2048 input capsules (n) across 8 cores. Each core builds
u_hat = einsum('bni,nio->bno') for its n-slice with W as the PE-stationary
operand so the PSUM output lands with (i4,j32) on partitions -- the native
layout for the routing b-update contraction over j. The o-contraction over n
runs on an n-partition view produced by DMA xbar transposes. Routing's
per-iteration global sum over n is an AllReduce of the tiny [32,32,32] o.

v2: dense host-side input layouts (contiguous DMA), o0 via upl-stationary
1024-col matmuls (64 instead of 512 PE instructions), b-update merged to
4-batches-per-stationary 1024-col matmuls (64 instead of 512 instructions
per iteration), bounce-free o scatter via DVE 32x32 transpose + partition-
shifted copies, per-8b-group softmax for PE/DVE pipelining, and a deeper
uht transpose ring so the o-pass xbar transposes prefetch during AllReduce
and b-update windows.
"""
import sys

sys.path.insert(0, "/opt/trn_rl_repo")

import numpy as np
import ml_dtypes

B = 32            # batch
N_TOTAL = 2048    # input capsules
KD = 16           # input capsule dim
NCAP = 32         # output capsules (i)
DIM = 32          # output capsule dim (j)
O = NCAP * DIM    # 1024
NUM_CORES = 8
NL = N_TOTAL // NUM_CORES   # 256 local n
G = NL // 8                 # 32 groups of 8 n
OC = O // 128               # 8 o-chunks
EPS_K = 1e-7
EPS_L2 = 1e-12

_PROG = {}


def _set_dims(ap, dims, offset=None):
    c = ap.copy()
    v = c.ap
    while len(v) > len(dims):
        v.pop()
    while len(v) < len(dims):
        v.insert(0, [0, 1])
    for k, d in enumerate(dims):
        v[k] = list(d)
    if offset is not None:
        c.offset = offset
    return c


def build_program(num_cores):
    import concourse.bass as bass
    import concourse.mybir as mybir
    from concourse import bacc, tile
    from concourse.tile import add_dep_helper

    f32 = mybir.dt.float32
    bf16 = mybir.dt.bfloat16
    AX = mybir.AxisListType
    OP = mybir.AluOpType
    AF = mybir.ActivationFunctionType

    nc = bacc.Bacc(
        "TRN2", target_bir_lowering=False, num_devices=num_cores,
        dynamic_dma_scratch_size=4096,
    )
    rg = [list(range(num_cores))]

    wl_d = nc.dram_tensor("wl", [OC, 128, G * 128], bf16, kind="ExternalInput")
    ubd_d = nc.dram_tensor("ubd", [128, G * 256], bf16, kind="ExternalInput")
    upl_d = nc.dram_tensor("upl", [128, G * B], bf16, kind="ExternalInput")
    out_d = nc.dram_tensor("out", [B, O], f32, kind="ExternalOutput")

    OBP = 8 * 8 * 128     # obd2 col pitch (8192): (b4, oc, 128)

    with tile.TileContext(nc) as tc:
        with (
            tc.tile_pool(name="pers", bufs=1) as pers,
            tc.tile_pool(name="dram", bufs=1, space="DRAM") as dram,
        ):
            u_hat = pers.tile([128, B, OC, NL], bf16, tag="u_hat")

            cc_in = [
                dram.tile([B, O], f32, tag=f"cc_in{t}", name=f"cc_in{t}")
                for t in range(3)
            ]
            cc_out = [
                dram.tile([B, O], f32, tag=f"cc_out{t}", name=f"cc_out{t}")
                for t in range(3)
            ]

            def all_reduce(t):
                if num_cores == 1:
                    nc.gpsimd.dma_start(cc_out[t][:], cc_in[t][:])
                else:
                    nc.gpsimd.collective_compute(
                        "AllReduce", OP.add, replica_groups=rg,
                        ins=[cc_in[t][:].opt()], outs=[cc_out[t][:].opt()],
                    )

            # ---------------- Phase A: load + build u_hat + o0 ----------------
            with (
                tc.tile_pool(name="apool", bufs=1) as apool,
                tc.tile_pool(name="ps_main", bufs=2, space="PSUM") as ps_main,
                tc.tile_pool(name="ps_o0", bufs=1, space="PSUM") as ps_o0,
            ):
                wl_sb = apool.tile([128, OC, G, 128], bf16, tag="wl_sb")
                ubd_sb = apool.tile([128, G, 256], bf16, tag="ubd_sb")
                upl_sb = apool.tile([128, G, B], bf16, tag="upl_sb")
                o0_sb = apool.tile([B, O], f32, tag="o0_sb")
                nc.sync.dma_start(
                    ubd_sb[:].rearrange("p g c -> p (g c)"), ubd_d[:]
                )
                nc.sync.dma_start(
                    upl_sb[:].rearrange("p g c -> p (g c)"), upl_d[:]
                )
                for oc in range(OC):
                    nc.sync.dma_start(
                        wl_sb[:, oc, :, :].rearrange("p g c -> p (g c)"),
                        wl_d[oc, :, :],
                    )

                for oc in range(OC):
                    for g4 in range(G // 4):
                        ps = ps_main.tile([128, 1024], f32, tag="ps")
                        for gh in range(4):
                            g = g4 * 4 + gh
                            nc.tensor.matmul(
                                ps[:, gh * 256:(gh + 1) * 256],
                                wl_sb[:, oc, g, :], ubd_sb[:, g, :],
                                start=True, stop=True,
                                skip_group_check=True,
                            )
                        # psum cols (gh, b, n8) -> u_hat[:, b, oc, g4*32+gh*8+n8]
                        g = g4 * 4
                        dst = _set_dims(
                            u_hat[:, :, oc, 0],
                            [[B * OC * NL, 128], [8, 4], [OC * NL, B], [1, 8]],
                            offset=oc * NL + g * 8,
                        )
                        src = ps[:].rearrange("p (h b n) -> p h b n", h=4, b=B)
                        if g4 % 2 == 0:
                            nc.scalar.copy(dst, src)
                        else:
                            nc.vector.tensor_copy(dst, src)

                # o0 = sum_n u_hat (softmax(0) scale folds into l2 normalize)
                o0a = ps_o0.tile([B, 512], f32, tag="o0a")
                o0b = ps_o0.tile([B, 512], f32, tag="o0b")
                for g in range(G):
                    nc.tensor.matmul(
                        o0a[:], upl_sb[:, g, :],
                        wl_sb[:, 0:4, g, :],
                        start=(g == 0), stop=(g == G - 1),
                        skip_group_check=True,
                    )
                    nc.tensor.matmul(
                        o0b[:], upl_sb[:, g, :],
                        wl_sb[:, 4:8, g, :],
                        start=(g == 0), stop=(g == G - 1),
                        skip_group_check=True,
                    )
                nc.scalar.copy(o0_sb[:, 0:512], o0a[:])
                nc.vector.tensor_copy(o0_sb[:, 512:1024], o0b[:])
                nc.sync.dma_start(cc_in[0][:], o0_sb[:])

            all_reduce(0)

            # ---------------- routing iterations ----------------
            with (
                tc.tile_pool(name="rout", bufs=1) as rout,
                tc.tile_pool(name="ring", bufs=3) as ring,
                tc.tile_pool(name="bu", bufs=2) as bu,
                tc.tile_pool(name="tring", bufs=2) as tring,
                tc.tile_pool(name="ps_b", bufs=2, space="PSUM") as ps_b,
                tc.tile_pool(name="ps_o", bufs=2, space="PSUM") as ps_o,
            ):
                tr_scr = rout.tile([128, 1024], f32, tag="tr_scr")
                o_acc = rout.tile([DIM, B, NCAP], f32, tag="o_acc")
                onrm = rout.tile([B, O], f32, tag="onrm")
                onl = rout.tile([B, O], f32, tag="onl")
                o_t = rout.tile([B, O], f32, tag="o_t")
                s2 = rout.tile([B, NCAP], f32, tag="s2")
                s2b = rout.tile([B, NCAP], f32, tag="s2b")
                s2c = rout.tile([B, NCAP], f32, tag="s2c")
                rinv = rout.tile([B, NCAP], f32, tag="rinv")
                obd2 = rout.tile([128, 8, 8, 128], bf16, tag="obd2")
                blog = rout.tile([128, B, 2, NCAP], f32, tag="blog")
                c_sb = rout.tile([128, B, 2, NCAP], bf16, tag="c_sb")
                mx = rout.tile([128, B, 2], f32, tag="mx")
                sm = rout.tile([128, B, 2], f32, tag="sm")
                smr = rout.tile([128, B, 2], f32, tag="smr")

                ms2 = nc.gpsimd.memset(obd2[:], 0.0)

                for t in range(3):
                    final = t == 2
                    ji = t > 0  # cc[0] is [b,(i,j)]; cc[1:] are [j,(b,i)]
                    # ---- load global o into [b, (i,j)] layout
                    if not ji:
                        nc.sync.dma_start(onrm[:], cc_out[t][:])
                    else:
                        # onrm[b, i*32+j] = cc_out[j, b*32+i]
                        dstL = onrm[:].rearrange("b (i j) -> b i j", i=NCAP)
                        srcL = _set_dims(
                            cc_out[t][:],
                            [[NCAP, B], [1, NCAP], [O, DIM]],
                        )
                        nc.sync.dma_start(dstL, srcL)
                    # ---- normalize (l2 for t<2, squash at t=2)
                    nc.scalar.square(onl[:], onrm[:])
                    nc.vector.tensor_reduce(
                        s2[:], onl[:].rearrange("b (i j) -> b i j", i=NCAP),
                        axis=AX.X, op=OP.add,
                    )
                    if not final:
                        nc.vector.tensor_scalar_max(s2b[:], s2[:], EPS_L2)
                        nc.scalar.sqrt(s2c[:], s2b[:])
                        nc.vector.reciprocal(rinv[:], s2c[:])
                    else:
                        # squash scale = s2 / ((1+s2) * sqrt(s2+eps))
                        nc.vector.tensor_scalar_add(s2b[:], s2[:], EPS_K)
                        nc.scalar.sqrt(s2b[:], s2b[:])
                        nc.vector.tensor_scalar_add(s2c[:], s2[:], 1.0)
                        nc.vector.tensor_mul(s2c[:], s2c[:], s2b[:])
                        nc.vector.reciprocal(s2b[:], s2c[:])
                        nc.vector.tensor_mul(rinv[:], s2b[:], s2[:])
                    sc_b = _set_dims(rinv[:], [[NCAP, B], [1, NCAP], [0, DIM]])
                    nc.vector.tensor_mul(
                        onl[:].rearrange("b (i j) -> b i j", i=NCAP),
                        onrm[:].rearrange("b (i j) -> b i j", i=NCAP),
                        sc_b,
                    )
                    if final:
                        nc.sync.dma_start(out_d[:], onl[:])
                        break

                    # ---- scatter normalized o into block-diag obd2 (bf16)
                    # o_t[j, i*32+b] = onl[b, i*32+j] (32x32 block transpose)
                    nc.vector.transpose(o_t[:], onl[:])
                    # obd2[32m+j, b4*1024+oc*132+bq*32+m] = o(b4*4+bq, oc*4+m, j)
                    for m in range(4):
                        src_e = _set_dims(
                            o_t[:],
                            [[O, DIM], [4, 8], [128, 8], [1, 4]],
                            offset=DIM * m,
                        )
                        dst_e = _set_dims(
                            obd2[:],
                            [[OBP, DIM], [1024, 8], [132, 8], [32, 4]],
                            offset=(32 * m) * OBP + m,
                        )
                        if m % 2 == 0:
                            ec = nc.vector.tensor_copy(dst_e, src_e)
                        else:
                            ec = nc.scalar.copy(dst_e, src_e)
                        if t == 0:
                            add_dep_helper(
                                ec.ins, ms2.ins, sync=True,
                                reason="mask copy after memset",
                            )

                    # ---- b-update: per b4 (4 batches), 8 oc x 2 half matmuls
                    # (512-col halves keep each matmul within one PSUM bank)
                    for b4 in range(8):
                        psb = ps_b.tile([128, 1024], f32, tag="psb")
                        for oc in range(OC):
                            for bh in range(2):
                                nc.tensor.matmul(
                                    psb[:, bh * 512:(bh + 1) * 512],
                                    obd2[:, b4, oc, :],
                                    u_hat[:, 4 * b4 + 2 * bh:4 * b4 + 2 * bh + 2,
                                          oc, :],
                                    start=(oc == 0), stop=(oc == OC - 1),
                                    skip_group_check=True,
                                )
                        sthi = bu.tile([128, 1024], bf16, tag="sthi")
                        stlo = bu.tile([128, 1024], bf16, tag="stlo")
                        nc.scalar.copy(sthi[:], psb[:])
                        nc.vector.tensor_sub(stlo[:], psb[:], sthi[:])
                        thi = tring.tile([128, 8, 128], bf16, tag="thi")
                        tlo = tring.tile([128, 8, 128], bf16, tag="tlo")
                        nc.sync.dma_start_transpose(thi[:], sthi[:])
                        nc.sync.dma_start_transpose(tlo[:], stlo[:])
                        # pick diag: blog[p, 4*b4+bq, nh, i] = t[p, bq*2+nh, bq*32+i]
                        thv = _set_dims(
                            thi[:], [[1024, 128], [288, 4], [128, 2], [1, 32]]
                        )
                        tlv = _set_dims(
                            tlo[:], [[1024, 128], [288, 4], [128, 2], [1, 32]]
                        )
                        nc.vector.tensor_add(
                            blog[:, 4 * b4:4 * (b4 + 1), :, :], thv, tlv
                        )

                        # ---- softmax over i, per 8-batch group
                        if b4 % 2 == 1:
                            g8 = b4 // 2
                            bl = blog[:, 8 * g8:8 * (g8 + 1), :, :]
                            bl3 = bl.rearrange("p b h i -> p (b h) i")
                            mxs = mx[:, 8 * g8:8 * (g8 + 1), :]
                            sms = sm[:, 8 * g8:8 * (g8 + 1), :]
                            smrs = smr[:, 8 * g8:8 * (g8 + 1), :]
                            nc.vector.tensor_reduce(
                                mxs, bl, axis=AX.X, op=OP.max
                            )
                            mxb = _set_dims(
                                mx[:, 0, 0],
                                [[2 * B, 128], [1, 16], [0, NCAP]],
                                offset=16 * g8,
                            )
                            nc.vector.tensor_sub(bl3, bl3, mxb)
                            nc.scalar.activation(bl, bl, AF.Exp)
                            nc.vector.tensor_reduce(
                                sms, bl, axis=AX.X, op=OP.add
                            )
                            nc.vector.reciprocal(smrs, sms)
                            smb = _set_dims(
                                smr[:, 0, 0],
                                [[2 * B, 128], [1, 16], [0, NCAP]],
                                offset=16 * g8,
                            )
                            nc.vector.tensor_mul(
                                c_sb[:, 8 * g8:8 * (g8 + 1), :, :].rearrange(
                                    "p b h i -> p (b h) i"
                                ),
                                bl3, smb,
                            )

                    # ---- o-pass: xbar-transpose u_hat per 2b, matmul with c
                    for b in range(B):
                        cg = b & 3
                        if cg == 0:
                            pso = ps_o.tile([128, 1024], f32, tag="pso")
                        if b % 2 == 0:
                            uht = ring.tile([128, 32, 128], bf16, tag="uht")
                            nc.sync.dma_start_transpose(
                                uht[:],
                                u_hat[:, b:b + 2, :, :].rearrange(
                                    "p b a n -> p (b a n)"
                                ),
                            )
                        b1 = b & 1
                        for nh in range(2):
                            lhs = c_sb[:, b, nh, :]
                            for oh in range(2):
                                rhs = _set_dims(
                                    uht[:],
                                    [[32 * 128, 128], [256, 4], [1, 128]],
                                    offset=(16 * b1 + 8 * oh + nh) * 128,
                                )
                                nc.tensor.matmul(
                                    pso[32 * cg:32 * cg + 32,
                                        oh * 512:(oh + 1) * 512],
                                    lhs, rhs,
                                    start=(nh == 0), stop=(nh == 1),
                                    tile_position=(0, 32 * cg),
                                    skip_group_check=True,
                                )
                        if cg == 3:
                            # 32x32 block transpose; diag becomes stride-33 cols
                            nc.vector.transpose(tr_scr[:], pso[:])
                            for c2 in range(4):
                                bb = b - 3 + c2
                                diag = _set_dims(
                                    tr_scr[:], [[1024, 32], [33, DIM]],
                                    offset=(32 * c2) * 1024,
                                )
                                if c2 % 2 == 0:
                                    nc.scalar.copy(o_acc[:, bb, :], diag)
                                else:
                                    nc.vector.tensor_copy(o_acc[:, bb, :], diag)

                    # o_acc [j, b, i] -> cc_in[t+1] (ji layout, same shape)
                    nc.sync.dma_start(cc_in[t + 1][:], o_acc[:])
                    all_reduce(t + 1)

    nc.compile()
    return nc


def host_prep(u_vecs, W, core):
    ns = slice(core * NL, (core + 1) * NL)
    Wc = np.asarray(W[ns], dtype=np.float32)             # [NL, 16, 1024]
    uc = np.asarray(u_vecs[:, ns, :], dtype=np.float32)  # [B, NL, 16]
    bf = ml_dtypes.bfloat16

    # wl[oc, n8*16+k, g*128+c] = W[g*8+n8, k, oc*128+c]
    wl = (
        Wc.reshape(G, 8, KD, OC, 128)
        .transpose(3, 1, 2, 0, 4)
        .reshape(OC, 128, G * 128)
        .astype(bf)
    )
    tmp = uc.transpose(1, 2, 0).reshape(G, 8, KD, B)     # [g, n8, k, b]
    # ubd[n8*16+k, g*256 + b*8+n8'] = u[b, g*8+n8, k] * (n8 == n8')
    ubd = np.zeros((8, KD, G, B, 8), dtype=np.float32)
    for n8 in range(8):
        ubd[n8, :, :, :, n8] = tmp[:, n8].transpose(1, 0, 2)
    ubd = ubd.reshape(128, G * B * 8).astype(bf)
    # upl[n8*16+k, g*32+b] = u[b, g*8+n8, k]
    upl = tmp.transpose(1, 2, 0, 3).reshape(128, G * B).astype(bf)
    return {"wl": wl, "ubd": ubd, "upl": upl}


def kernel(u_vecs, W):
    from concourse import bass_utils

    if "prog" not in _PROG:
        _PROG["prog"] = build_program(NUM_CORES)
    nc = _PROG["prog"]
    in_maps = [host_prep(u_vecs, W, c) for c in range(NUM_CORES)]
    res = bass_utils.run_bass_kernel_spmd(
        nc, in_maps, core_ids=list(range(NUM_CORES))
    )
    out = np.asarray(res.results[0]["out"], dtype=np.float32)
    return out.reshape(B, NCAP, DIM)
